# revision 1
# baseline (speedup 1.0000x reference)
"""Block-causal self-attention (SSMax) Trainium2 kernel.

Full inputs in, full output out. Sharding: 8 cores = 2 batches x 4 head
groups (3 heads each). Each core computes qkv for its head slice, the
block-causal attention for its 3 heads, and a partial c_proj product;
the host sums the 4 partials per batch.

Device-side layout notes (per core):
  - x is shipped pre-transposed and pre-cast: xt [768, 2048] bf16 so the
    tensor engine contracts over channels (K=partition) with natural DMA
    layouts and half the HBM traffic of fp32.
  - c_attn slice shipped as wqkv [768, 576] bf16, column order
    [q_h0*, k_h0, q_h1*, k_h1, q_h2*, k_h2, v_h0, v_h1, v_h2] (64 cols
    each); q columns pre-scaled by s*log(T)/sqrt(hd) so softmax scaling
    is free.
  - The qkv projection, k/v partition shifts, zero-padding and v
    transposes are all staged per 512-token range so attention group ci
    (which only needs token ranges <= ci) pipelines under the qkv tail.
  - Scores are computed transposed (ST[j, i] = k_j . q_i) so the exp'd
    tile is directly the K-side operand of the P@V matmul.
  - P@V runs with V as the stationary operand, producing yT[e, i]
    (features on partitions) directly: 512-col streams with one weight
    load per (jc, head) instead of one per 128-query chunk, and no
    y transposes before the projection.
  - The softmax denominator comes from an extra ones-column appended to
    V (feature row 64 of the yT psum). yT rows are normalized with
    reciprocal_approx_fast + a gpsimd partition-broadcast + one DVE
    multiply per (group, head).
  - Softmax skips the max-subtraction pass: scores are ~N(0,1) for this
    problem so exp is fp32/bf16-safe.
"""

import numpy as np

T = 2048
C = 768
HEADS_PER_CORE = 3
HD = 64
NBLK = 64  # block-causal block size
KC = 6  # 768 / 128 contraction chunks
N_CORES = 8

_CACHE: dict = {}


def _build_bass():
    import concourse.bacc as bacc
    import concourse.mybir as mybir
    import concourse.tile as tile
    from concourse._compat import get_trn_type
    from concourse.masks import make_identity

    dt = mybir.dt
    f32 = dt.float32
    f32r = dt.float32r
    bf16 = dt.bfloat16
    EXP = mybir.ActivationFunctionType.Exp
    LN = mybir.ActivationFunctionType.Ln
    MUL = mybir.AluOpType.mult

    nc = bacc.Bacc(get_trn_type() or "TRN2", debug=False)
    xt_d = nc.dram_tensor("xt", [C, T], bf16, kind="ExternalInput")
    wqkv_d = nc.dram_tensor("wqkv", [C, 576], bf16, kind="ExternalInput")
    wproj_d = nc.dram_tensor("wproj", [256, C], bf16, kind="ExternalInput")
    out_d = nc.dram_tensor("out", [T, C], f32, kind="ExternalOutput")
    warm_d = nc.dram_tensor("warm", [128, 1], f32, kind="ExternalOutput")

    with tile.TileContext(nc) as tc:
        with (
            tc.tile_pool(name="persist", bufs=1) as persist,
            tc.tile_pool(name="ps_big", bufs=2, space="PSUM") as ps_big,
            tc.tile_pool(name="ps_st", bufs=2, space="PSUM") as ps_st,
            tc.tile_pool(name="ps_y", bufs=2, space="PSUM") as ps_y,
            tc.tile_pool(name="exp_pool", bufs=2) as exp_pool,
            tc.tile_pool(name="small", bufs=4) as small,
            tc.tile_pool(name="outst", bufs=3) as outst,
        ):
            xt_all = persist.tile([128, KC, T], bf16, tag="xt")
            w_all = persist.tile([128, KC, 576], bf16, tag="w")
            wp_all = persist.tile([128, 2, C], bf16, tag="wp")
            # wqkv column order (64 each): [q0,k0 | q1,k1 | q2,k2 | v0,v1 | v2].
            # The PE crashes if consecutive instructions use different base
            # partitions, so everything it touches is staged at base 0:
            # k_h and v1 are shifted down with SBUF->SBUF DMAs after the
            # qkv projection.
            qk0 = persist.tile([128, T], bf16, tag="qk0")  # [q0; k0]
            qk1 = persist.tile([128, T], bf16, tag="qk1")  # [q1; k1]
            qk2 = persist.tile([128, T], bf16, tag="qk2")  # [q2; k2]
            vst = persist.tile([128, T], bf16, tag="vst")  # [v0; v1]
            v2st = persist.tile([64, T], bf16, tag="v2")  # [v2]
            # k goes to rows 0:64 of its own tile; rows 64:128 of both the
            # k tiles and the q tiles are zeroed so score matmuls run with
            # K=128 (K=64 matmuls serialize LDWEIGHTS, costing 2x)
            kt0 = persist.tile([128, T], bf16, tag="kt0")
            kt1 = persist.tile([128, T], bf16, tag="kt1")
            kt2 = persist.tile([128, T], bf16, tag="kt2")
            v1t = persist.tile([64, T], bf16, tag="v1t")
            v_all = persist.tile([128, 16, 195], bf16, tag="v")
            # yT staging for the projection: slot 0 = features 0:128
            # (heads 0,1), slot 1 rows 0:64 = head 2. Slot-1 rows 64:128
            # are garbage but the matching wproj rows are host-zeroed.
            yt_all = persist.tile([128, 2, T], bf16, tag="yt")
            id_bf = persist.tile([128, 128], bf16, tag="idb")
            id_f = persist.tile([128, 128], f32, tag="idf")

            make_identity(nc, id_bf)
            make_identity(nc, id_f)
            # kt rows 64:128 and yt slot-1 rows 64:128 are never written;
            # zero them once during the DMA prologue while Pool is idle.
            # (yt garbage would be multiplied by the zero wproj rows, but
            # stale NaN bit patterns poison the product: NaN * 0 = NaN.)
            for t_ in (kt0, kt1, kt2):
                nc.gpsimd.memset(t_[64:128, :], 0.0)
            nc.gpsimd.memset(yt_all[64:128, 1, :], 0.0)

            # ---- PE warm-up: dense dummy matmuls during the DMA prologue
            # keep the HAM clock-gate open so qkv starts at 2.4 GHz ----
            wsink = persist.tile([128, 1], f32, tag="wsink")
            for wi in range(60):
                pw = ps_big.tile([128, 512], f32, tag="ps")
                nc.tensor.matmul(
                    pw[:, 0:128], lhsT=id_bf[:, :], rhs=id_bf[:, :],
                    start=True, stop=True,
                )
                if wi == 59:
                    nc.vector.tensor_copy(out=wsink[:, :], in_=pw[:, 0:1])
            nc.sync.dma_start(out=warm_d[:, :], in_=wsink[:, :])

            # ---- loads ----
            for kc in range(KC):
                nc.sync.dma_start(
                    out=w_all[:, kc, :], in_=wqkv_d[128 * kc : 128 * kc + 128, :]
                )
            # wproj is host-padded to 256 rows (rows 192:256 zero) so both
            # slots DMA straight in; the zero rows cancel the garbage rows
            # 64:128 of yt slot 1 in the projection matmul
            nc.sync.dma_start(out=wp_all[:, 0, :], in_=wproj_d[0:128, :])
            nc.sync.dma_start(out=wp_all[:, 1, :], in_=wproj_d[128:256, :])
            # x as six full-row DMAs fanned across three issuing engines so
            # the HWDGE queues run in parallel (a single issuer's queue only
            # sustains ~half the per-core HBM bandwidth). The issuing engines
            # have no pending deps here so the issue itself is free.
            xt_issuers = [nc.sync, nc.scalar]
            for kc in range(KC):
                xt_issuers[kc % 2].dma_start(
                    out=xt_all[:, kc, :],
                    in_=xt_d[128 * kc : 128 * kc + 128, :],
                )

            # ---- qkv projection + shifts + zero-pads + v transposes,
            # all per 512-token range so attention can pipeline in ----
            qkv_dst = [qk0, qk1, qk2, vst, v2st]
            for t4 in range(4):
                ts = slice(512 * t4, 512 * t4 + 512)
                for m in range(5):
                    rows = 128 if m < 4 else 64
                    ps = ps_big.tile([128, 512], f32, tag="ps")
                    for kc in range(KC):
                        nc.tensor.matmul(
                            ps[0:rows, :],
                            lhsT=w_all[:, kc, 128 * m : 128 * m + rows],
                            rhs=xt_all[:, kc, ts],
                            start=(kc == 0),
                            stop=(kc == KC - 1),
                        )
                    nc.vector.tensor_copy(
                        out=qkv_dst[m][0:rows, ts], in_=ps[0:rows, :]
                    )
                # shift k_h / v1 of this range to base partition 0
                # (SBUF->SBUF DMA), then zero-pad the score operands' rows
                for qk_t, kt_t in ((qk0, kt0), (qk1, kt1), (qk2, kt2)):
                    nc.sync.dma_start(out=kt_t[0:64, ts], in_=qk_t[64:128, ts])
                    nc.gpsimd.memset(qk_t[64:128, ts], 0.0)
                nc.sync.dma_start(out=v1t[0:64, ts], in_=vst[64:128, ts])

            # ---- attention, group (ci) outer / head inner; group ci only
            # depends on token ranges <= ci so it overlaps the qkv tail,
            # and the projection of group ci overlaps group ci+1 ----
            head_ops = [
                (kt0, qk0),
                (kt1, qk1),
                (kt2, qk2),
            ]
            def emit_proj(pci):
                """Projection for group pci's 4 t-chunks. Emitted midway
                through the NEXT group's head loop so the PE's in-order
                stream has score work covering the normalize chain's
                cross-engine latency."""
                for r in range(4):
                    tcn = 4 * pci + r
                    tsl = slice(128 * tcn, 128 * tcn + 128)
                    ot = outst.tile([128, C], f32, tag="ot")
                    # proj psum comes from the qkv-phase ring (idle during
                    # attention apart from the v transposes)
                    pp = ps_big.tile([128, 512], f32, tag="ps")
                    for ch in range(2):
                        nc.tensor.matmul(
                            pp[:, 0:512],
                            lhsT=yt_all[:, ch, tsl],
                            rhs=wp_all[:, ch, 0:512],
                            start=(ch == 0),
                            stop=(ch == 1),
                        )
                    nc.vector.tensor_copy(out=ot[:, 0:512], in_=pp[:, 0:512])
                    pp2 = ps_big.tile([128, 512], f32, tag="ps")
                    for ch in range(2):
                        nc.tensor.matmul(
                            pp2[:, 0:256],
                            lhsT=yt_all[:, ch, tsl],
                            rhs=wp_all[:, ch, 512:768],
                            start=(ch == 0),
                            stop=(ch == 1),
                        )
                    # Copy shares the Exp activation table: no table reload
                    nc.scalar.copy(out=ot[:, 512:768], in_=pp2[:, 0:256])
                    nc.sync.dma_start(out=out_d[tsl, :], in_=ot[:, :])

            for ci in range(4):
                i_base = 512 * ci
                # v transpose into [token, head-dim] layout + ones column.
                # Emitted here (not in the qkv loop) so the PE's in-order
                # stream doesn't stall on the v1 shift-DMA round trip: by
                # the time group ci's block runs, range ci's shift is long
                # done. Allocated from the qkv-phase psum ring, NOT ps_st:
                # sharing the score-pair ring would serialize attention
                # behind v transposes.
                for tcn in range(4 * ci, 4 * ci + 4):
                    tsl = slice(128 * tcn, 128 * tcn + 128)
                    pv = ps_big.tile([128, 192], bf16, tag="ps")
                    nc.tensor.transpose(
                        pv[:, 0:64], vst[0:64, tsl], id_bf[0:64, 0:64]
                    )
                    nc.tensor.transpose(
                        pv[:, 64:128], v1t[0:64, tsl], id_bf[0:64, 0:64]
                    )
                    nc.tensor.transpose(
                        pv[:, 128:192], v2st[0:64, tsl], id_bf[0:64, 0:64]
                    )
                    vdst = v_all[:, tcn, :].rearrange("p (h e) -> p h e", e=65)
                    nc.vector.tensor_copy(
                        out=vdst[:, :, 0:64],
                        in_=pv[:, 0:192].rearrange("p (h e) -> p h e", e=64),
                    )
                    nc.vector.memset(vdst[:, :, 64:65], 1.0)
                # per-group denominator staging: 3 rows collected, then one
                # batched transpose -> cheap [128, 12] reciprocal
                dsb = small.tile([3, 512], f32, tag="dsb", bufs=2)
                ysb = []
                for h in range(HEADS_PER_CORE):
                    k_sl, q_sl = head_ops[h]
                    # score tiles in pairs of j-chunks: two matmuls into one
                    # 2-bank psum, one wide exp (halves ACT instruction count)
                    ets = {}
                    npair = 2 * ci + 2
                    for p in range(npair):
                        ps = ps_st.tile([128, 1024], f32, tag="st")
                        et = exp_pool.tile([128, 1024], bf16, tag=f"p{p}")
                        exp_from = None  # start col of a pending fused exp
                        for half in range(2):
                            jc = 2 * p + half
                            m = jc - 4 * ci
                            i0 = 128 * m if m >= 0 else 0
                            lo = 512 * half
                            nc.tensor.matmul(
                                ps[:, lo + i0 : lo + 512],
                                lhsT=k_sl[:, 128 * jc : 128 * jc + 128],
                                rhs=q_sl[:, i_base + i0 : i_base + 512],
                                start=True,
                                stop=True,
                            )  # K=128 with zero-padded rows 64:128
                            if i0 == 0 and half == 0:
                                exp_from = 0  # may fuse with second half
                            elif i0 == 0 and exp_from == 0:
                                pass  # second half contiguous with first
                            else:
                                if exp_from is not None:
                                    nc.scalar.activation(
                                        et[:, exp_from:lo], ps[:, exp_from:lo], EXP
                                    )
                                exp_from = lo + i0
                            ets[jc] = et
                        nc.scalar.activation(
                            et[:, exp_from:1024], ps[:, exp_from:1024], EXP
                        )
                        for half in range(2):
                            jc = 2 * p + half
                            m = jc - 4 * ci
                            if m >= 0:
                                i0 = 512 * half + 128 * m
                                # block-causal: upper half-block keys masked
                                # for lower half-block queries (DVE: putting
                                # these on Pool makes its in-order queue sit
                                # in waits that delay later Pool work)
                                nc.vector.memset(et[64:128, i0 : i0 + 64], 0.0)

                    if h == 1 and ci > 0:
                        emit_proj(ci - 1)

                    # ---- P@V with V stationary: yT[e, i] accumulated over
                    # j-chunks, one 512-col stream per (jc, head). Feature
                    # row 64 is the softmax denominator (ones column) ----
                    py = ps_y.tile([128, 512], f32, tag="py")
                    last = 4 * ci + 3
                    for jc in range(last + 1):
                        m = jc - 4 * ci
                        i0 = 128 * m if m >= 0 else 0
                        lo = 512 * (jc & 1)
                        nc.tensor.matmul(
                            py[0:65, i0:512],
                            lhsT=v_all[:, jc, 65 * h : 65 * h + 65],
                            rhs=ets[jc][:, lo + i0 : lo + 512],
                            start=(jc == 0),
                            stop=(jc == last),
                        )

                    # stage this head's unnormalized yT + denominator row to
                    # SBUF immediately so the psum tile can recycle (the
                    # normalize below waits for all 3 heads). The denominator
                    # row then partition-shifts into the collection tile via
                    # SBUF->SBUF DMA (DVE copies cannot cross lanes).
                    ys = small.tile([65, 512], f32, tag="ysb", bufs=4)
                    nc.vector.tensor_copy(out=ys, in_=py[0:65, 0:512])
                    ysb.append(ys)
                    nc.sync.dma_start(
                        out=dsb[h : h + 1, :], in_=ys[64:65, :]
                    )

                # ---- normalize: yT[e, i] *= 1/denom[i]. A [1, 512] row
                # reciprocal on the DVE costs ~3.4us (iterative divide is
                # ~8 cyc/elem), so transpose the 3 denominator rows to
                # columns, take one cheap [128, 12] reciprocal, and
                # transpose back. Broadcast across the 64 feature rows is
                # a gpsimd partition-broadcast (the DVE multiply may read
                # only one PSUM operand, so everything lands in SBUF) ----
                dT = ps_y.tile([128, 4, 3], f32, tag="py")
                for c in range(4):
                    nc.tensor.transpose(
                        dT[:, c, 0:3], dsb[0:3, 128 * c : 128 * c + 128],
                        id_f[0:3, 0:3],
                    )
                rct = small.tile([128, 4, 3], f32, tag="rct", bufs=2)
                nc.vector.reciprocal(rct, dT[:, :, :])
                # per-head back-transposes: partition_broadcast (and DVE
                # lane alignment) require each reciprocal row to start at
                # partition 0 of its own tile
                rts = []
                for h in range(HEADS_PER_CORE):
                    rTh = ps_y.tile([1, 512], f32, tag="py")
                    for c in range(4):
                        nc.tensor.transpose(
                            rTh[0:1, 128 * c : 128 * c + 128],
                            rct[:, c, h : h + 1],
                            id_f[0:128, 0:128],
                        )
                    rs = small.tile([1, 512], f32, tag="rts", bufs=4)
                    nc.vector.tensor_copy(out=rs, in_=rTh[0:1, :])
                    rts.append(rs)

                isl = slice(i_base, i_base + 512)
                for h in range(HEADS_PER_CORE):
                    brc = small.tile([64, 512], f32, tag="brc")
                    nc.gpsimd.partition_broadcast(brc, rts[h])
                    if h == 0:
                        nc.vector.tensor_tensor(
                            out=yt_all[0:64, 0, isl],
                            in0=ysb[h][0:64, :], in1=brc, op=MUL,
                        )
                    elif h == 2:
                        nc.vector.tensor_tensor(
                            out=yt_all[0:64, 1, isl],
                            in0=ysb[h][0:64, :], in1=brc, op=MUL,
                        )
                    else:
                        # head 1's features live on partitions 64:128 of
                        # yt slot 0: normalize into a staging tile, then
                        # partition-shift with an SBUF->SBUF DMA
                        h1t = small.tile([64, 512], bf16, tag="h1t")
                        nc.vector.tensor_tensor(
                            out=h1t, in0=ysb[h][0:64, :], in1=brc, op=MUL,
                        )
                        nc.sync.dma_start(out=yt_all[64:128, 0, isl], in_=h1t)

            emit_proj(3)

    nc.compile()
    return nc


def _get_nc():
    if "nc" not in _CACHE:
        _CACHE["nc"] = _build_bass()
    return _CACHE["nc"]


def make_in_maps(x, c_attn_w, c_proj_w, s):
    import ml_dtypes

    bf16 = ml_dtypes.bfloat16
    x = np.asarray(x, dtype=np.float32)
    c_attn_w = np.asarray(c_attn_w, dtype=np.float32)
    c_proj_w = np.asarray(c_proj_w, dtype=np.float32)
    s = np.asarray(s, dtype=np.float32)

    scale = np.float32(s[0] * np.log(T).astype(np.float32))
    f = np.float32(scale * np.float32(1.0 / np.sqrt(HD)))

    in_maps = []
    for b in range(2):
        xt = np.ascontiguousarray(x[b].T).astype(bf16)  # [768, 2048]
        for g in range(4):
            h0, h1, h2 = 3 * g, 3 * g + 1, 3 * g + 2
            qrow = lambda h: c_attn_w[64 * h : 64 * h + 64] * f  # scaled q
            krow = lambda h: c_attn_w[C + 64 * h : C + 64 * h + 64]
            vrow = lambda h: c_attn_w[2 * C + 64 * h : 2 * C + 64 * h + 64]
            # column order [q0,k0 | q1,k1 | q2,k2 | v0,v1 | v2] (see device side)
            wsel = np.concatenate(
                [
                    qrow(h0), krow(h0),
                    qrow(h1), krow(h1),
                    qrow(h2), krow(h2),
                    vrow(h0), vrow(h1),
                    vrow(h2),
                ],
                axis=0,
            )  # [576, 768]
            wqkv = np.ascontiguousarray(wsel.T).astype(bf16)  # [768, 576]
            wproj = np.zeros((256, C), np.float32)  # rows 192:256 stay zero
            wproj[0:192] = c_proj_w[:, 192 * g : 192 * g + 192].T
            in_maps.append(
                {"xt": xt, "wqkv": wqkv, "wproj": wproj.astype(bf16)}
            )
    return in_maps


def gather(results):
    out = np.empty((2, T, C), dtype=np.float32)
    for b in range(2):
        acc = results[4 * b]["out"].astype(np.float32)
        for g in range(1, 4):
            acc = acc + results[4 * b + g]["out"]
        out[b] = acc
    return out


def kernel(x, c_attn_w, c_proj_w, s):
    from concourse.bass_utils import run_bass_kernel_spmd

    nc = _get_nc()
    in_maps = make_in_maps(x, c_attn_w, c_proj_w, s)
    res = run_bass_kernel_spmd(nc, in_maps, list(range(N_CORES)))
    return gather(res.results)



# revision 4
# speedup vs baseline: 1.1719x; 1.1719x over previous
"""Block-causal self-attention (SSMax) Trainium2 kernel.

Full inputs in, full output out. Sharding: 8 cores = 2 batches x 4 head
groups (3 heads each). Each core computes qkv for its head slice, the
block-causal attention for its 3 heads, and a partial c_proj product;
the host sums the 4 partials per batch.

Device-side layout notes (per core):
  - x is shipped pre-transposed and pre-cast: xt [768, 2048] bf16 so the
    tensor engine contracts over channels (K=partition) with natural DMA
    layouts and half the HBM traffic of fp32.
  - c_attn slice shipped as wqkv [768, 576] bf16, column order
    [q_h0*, k_h0, q_h1*, k_h1, q_h2*, k_h2, v_h0, v_h1, v_h2] (64 cols
    each); q columns pre-scaled by s*log(T)/sqrt(hd) so softmax scaling
    is free.
  - Input DMAs are ordered for pipelining: wqkv first (needed by every
    qkv matmul), then xt in token-quarter-major order so the t4=0 qkv
    can start while the rest of x streams in; wproj (needed only by the
    projection) last.  A gapless PE warm-up stream (single psum tile,
    no pool cycling) covers the DMA prologue so the HAM clock gate is
    open and the qkv matmuls run at 2.4 GHz from the first instruction.
  - The qkv projection, k/v partition shifts, zero-padding and v
    transposes are all staged per 512-token range so attention group ci
    (which only needs token ranges <= ci) pipelines under the qkv tail.
  - Scores are computed transposed (ST[j, i] = k_j . q_i) so the exp'd
    tile is directly the K-side operand of the P@V matmul.
  - P@V runs with V as the stationary operand, producing yT[e, i]
    (features on partitions) directly: 512-col streams with one weight
    load per (jc, head) instead of one per 128-query chunk, and no
    y transposes before the projection.
  - The softmax denominator comes from an extra ones-column appended to
    V (feature row 64 of the yT psum). Normalization: the denominator
    row is partition-shifted to a [1, 512] tile, broadcast across 64
    partitions with a K=1 f32r outer-product matmul (213 ns on the PE,
    replacing a ~1 us gpsimd partition-broadcast), reciprocal'd with
    the fast DVE approx, and multiplied into yT.  The normalize for
    group ci is emitted inside group ci+1's head loop (like the
    projection) so its cross-engine latency hides under score work.
  - Softmax skips the max-subtraction pass: scores are ~N(0,1) for this
    problem so exp is fp32/bf16-safe.
"""

import numpy as np

T = 2048
C = 768
HEADS_PER_CORE = 3
HD = 64
NBLK = 64  # block-causal block size
KC = 6  # 768 / 128 contraction chunks
N_CORES = 8

_CACHE: dict = {}


def _build_bass():
    import concourse.bacc as bacc
    import concourse.mybir as mybir
    import concourse.tile as tile
    from concourse._compat import get_trn_type
    from concourse.masks import make_identity

    dt = mybir.dt
    f32 = dt.float32
    f32r = dt.float32r
    bf16 = dt.bfloat16
    EXP = mybir.ActivationFunctionType.Exp
    MUL = mybir.AluOpType.mult

    nc = bacc.Bacc(get_trn_type() or "TRN2", debug=False)
    xt_d = nc.dram_tensor("xt", [C, T], bf16, kind="ExternalInput")
    wqkv_d = nc.dram_tensor("wqkv", [C, 576], bf16, kind="ExternalInput")
    wproj_d = nc.dram_tensor("wproj", [256, C], bf16, kind="ExternalInput")
    out_d = nc.dram_tensor("out", [T, C], f32, kind="ExternalOutput")
    warm_d = nc.dram_tensor("warm", [128, 1], f32, kind="ExternalOutput")

    with tile.TileContext(nc) as tc:
        with (
            tc.tile_pool(name="persist", bufs=1) as persist,
            tc.tile_pool(name="ps_big", bufs=2, space="PSUM") as ps_big,
            tc.tile_pool(name="ps_st", bufs=2, space="PSUM") as ps_st,
            tc.tile_pool(name="ps_y", bufs=2, space="PSUM") as ps_y,
            tc.tile_pool(name="exp_pool", bufs=2) as exp_pool,
            tc.tile_pool(name="small", bufs=4) as small,
            tc.tile_pool(name="outst", bufs=3) as outst,
        ):
            xt_all = persist.tile([128, KC, T], bf16, tag="xt")
            w_all = persist.tile([128, KC, 576], bf16, tag="w")
            wp_all = persist.tile([128, 2, C], bf16, tag="wp")
            # wqkv column order (64 each): [q0,k0 | q1,k1 | q2,k2 | v0,v1 | v2].
            # The PE crashes if consecutive instructions use different base
            # partitions, so everything it touches is staged at base 0:
            # k_h and v1 are shifted down with SBUF->SBUF DMAs after the
            # qkv projection.
            qk0 = persist.tile([128, T], bf16, tag="qk0")  # [q0; k0]
            qk1 = persist.tile([128, T], bf16, tag="qk1")  # [q1; k1]
            qk2 = persist.tile([128, T], bf16, tag="qk2")  # [q2; k2]
            vst = persist.tile([128, T], bf16, tag="vst")  # [v0; v1]
            v2st = persist.tile([64, T], bf16, tag="v2")  # [v2]
            # k goes to rows 0:64 of its own tile; rows 64:128 of both the
            # k tiles and the q tiles are zeroed so score matmuls run with
            # K=128 (K=64 matmuls serialize LDWEIGHTS, costing 2x)
            kt0 = persist.tile([128, T], bf16, tag="kt0")
            kt1 = persist.tile([128, T], bf16, tag="kt1")
            kt2 = persist.tile([128, T], bf16, tag="kt2")
            v1t = persist.tile([64, T], bf16, tag="v1t")
            v_all = persist.tile([128, 16, 195], bf16, tag="v")
            # yT staging for the projection: slot 0 = features 0:128
            # (heads 0,1), slot 1 rows 0:64 = head 2. Slot-1 rows 64:128
            # are garbage but the matching wproj rows are host-zeroed.
            yt_all = persist.tile([128, 2, T], bf16, tag="yt")
            id_bf = persist.tile([128, 128], bf16, tag="idb")
            ones_f = persist.tile([1, 64], f32, tag="ones")

            make_identity(nc, id_bf)
            nc.vector.memset(ones_f[:, :], 1.0)
            # kt rows 64:128 and yt slot-1 rows 64:128 are never written;
            # zero them once during the DMA prologue (split DVE/gpsimd so
            # neither queue serializes behind all four memsets).
            # (yt garbage would be multiplied by the zero wproj rows, but
            # stale NaN bit patterns poison the product: NaN * 0 = NaN.)
            nc.gpsimd.memset(kt0[64:128, :], 0.0)
            nc.gpsimd.memset(kt1[64:128, :], 0.0)
            nc.vector.memset(kt2[64:128, :], 0.0)
            nc.vector.memset(yt_all[64:128, 1, :], 0.0)

            # ---- loads: wqkv first (every qkv matmul needs it), then xt
            # in token-quarter-major order so qkv t4=0 can start while the
            # rest of x streams; wproj (projection-only) last. Two issuing
            # engines because a single HWDGE queue only sustains ~half the
            # per-core HBM bandwidth. ----
            issuers = [nc.sync, nc.scalar]
            out_issuers = [nc.sync, nc.gpsimd]
            for kc in range(KC):
                issuers[kc % 2].dma_start(
                    out=w_all[:, kc, :], in_=wqkv_d[128 * kc : 128 * kc + 128, :]
                )
            for t4 in range(4):
                ts_ = slice(512 * t4, 512 * t4 + 512)
                for kc in range(KC):
                    issuers[(kc + t4) % 2].dma_start(
                        out=xt_all[:, kc, ts_],
                        in_=xt_d[128 * kc : 128 * kc + 128, ts_],
                    )
            # wproj is host-padded to 256 rows (rows 192:256 zero) so both
            # slots DMA straight in; the zero rows cancel the garbage rows
            # 64:128 of yt slot 1 in the projection matmul
            nc.sync.dma_start(out=wp_all[:, 0, :], in_=wproj_d[0:128, :])
            nc.scalar.dma_start(out=wp_all[:, 1, :], in_=wproj_d[128:256, :])

            # ---- PE warm-up: a gapless stream of matmuls into a single
            # psum tile (no pool cycling => no semaphore waits between
            # them) keeps the PE busy until the t4=0 inputs land AND gives
            # the HAM >3us of continuous work so qkv starts at 2.4 GHz ----
            wsink = persist.tile([128, 1], f32, tag="wsink")
            warm_rhs = persist.tile([128, 512], bf16, tag="wrhs")
            nc.gpsimd.memset(warm_rhs[:, :], 0.0)
            pw = ps_big.tile([128, 512], f32, tag="ps")
            for wi in range(26):
                nc.tensor.matmul(
                    pw[:, :], lhsT=id_bf[:, :], rhs=warm_rhs[:, :],
                    start=True, stop=True,
                )
            nc.vector.tensor_copy(out=wsink[:, :], in_=pw[:, 0:1])
            nc.sync.dma_start(out=warm_d[:, :], in_=wsink[:, :])

            # ---- qkv projection + shifts + zero-pads, per 512-token
            # range so attention can pipeline in ----
            qkv_dst = [qk0, qk1, qk2, vst, v2st]
            for t4 in range(4):
                ts = slice(512 * t4, 512 * t4 + 512)
                for m in range(5):
                    rows = 128 if m < 4 else 64
                    ps = ps_big.tile([128, 512], f32, tag="ps")
                    for kc in range(KC):
                        nc.tensor.matmul(
                            ps[0:rows, :],
                            lhsT=w_all[:, kc, 128 * m : 128 * m + rows],
                            rhs=xt_all[:, kc, ts],
                            start=(kc == 0),
                            stop=(kc == KC - 1),
                        )
                    nc.vector.tensor_copy(
                        out=qkv_dst[m][0:rows, ts], in_=ps[0:rows, :]
                    )
                # shift k_h / v1 of this range to base partition 0
                # (SBUF->SBUF DMA), then zero-pad the score operands' rows
                for qk_t, kt_t in ((qk0, kt0), (qk1, kt1), (qk2, kt2)):
                    nc.sync.dma_start(out=kt_t[0:64, ts], in_=qk_t[64:128, ts])
                    nc.gpsimd.memset(qk_t[64:128, ts], 0.0)
                nc.sync.dma_start(out=v1t[0:64, ts], in_=vst[64:128, ts])

            # ---- attention, group (ci) outer / head inner; group ci only
            # depends on token ranges <= ci so it overlaps the qkv tail,
            # and the normalize+projection of group ci overlap group ci+1 ----
            head_ops = [
                (kt0, qk0),
                (kt1, qk1),
                (kt2, qk2),
            ]
            ysb_all = {}  # (ci, h) -> unnormalized yT staging tile
            rd_all = {}  # (ci, h) -> [1, 512] denominator row tile

            def emit_normalize(pci):
                """Normalize group pci's yT rows: yt[e,i] = ys[e,i]/d[i].
                d rows were partition-shifted to [1,512] tiles during the
                PV stage; here each is broadcast across 64 partitions with
                a K=1 f32r outer-product matmul, reciprocal'd (fast DVE
                approx, ~18 bits), and multiplied into the yt staging
                tiles. Emitted inside the NEXT group's head loop so the
                chain's cross-engine latency hides under score work."""
                isl = slice(512 * pci, 512 * pci + 512)
                for h in range(HEADS_PER_CORE):
                    brcd = ps_y.tile([128, 512], f32, tag="py")
                    nc.tensor.matmul(
                        brcd[0:64, :],
                        lhsT=ones_f[0:1, :].bitcast(f32r),
                        rhs=rd_all[(pci, h)][0:1, :].bitcast(f32r),
                        start=True,
                        stop=True,
                    )
                    rcb = small.tile([64, 512], f32, tag="rcb", bufs=2)
                    nc.vector.reciprocal_approx_fast(rcb, brcd[0:64, :])
                    ys = ysb_all.pop((pci, h))
                    if h == 0:
                        nc.vector.tensor_tensor(
                            out=yt_all[0:64, 0, isl],
                            in0=ys[0:64, :], in1=rcb, op=MUL,
                        )
                    elif h == 2:
                        nc.vector.tensor_tensor(
                            out=yt_all[0:64, 1, isl],
                            in0=ys[0:64, :], in1=rcb, op=MUL,
                        )
                    else:
                        # head 1's features live on partitions 64:128 of
                        # yt slot 0: normalize into a staging tile, then
                        # partition-shift with an SBUF->SBUF DMA
                        h1t = small.tile([64, 512], bf16, tag="h1t")
                        nc.vector.tensor_tensor(
                            out=h1t, in0=ys[0:64, :], in1=rcb, op=MUL,
                        )
                        nc.sync.dma_start(out=yt_all[64:128, 0, isl], in_=h1t)

            def emit_proj(pci):
                """Projection for group pci's 4 t-chunks. Emitted midway
                through the NEXT group's head loop so the PE's in-order
                stream has score work covering the normalize chain's
                cross-engine latency."""
                for r in range(4):
                    tcn = 4 * pci + r
                    tsl = slice(128 * tcn, 128 * tcn + 128)
                    ot = outst.tile([128, C], f32, tag="ot")
                    # proj psum comes from the qkv-phase ring (idle during
                    # attention apart from the v transposes)
                    pp = ps_big.tile([128, 512], f32, tag="ps")
                    for ch in range(2):
                        nc.tensor.matmul(
                            pp[:, 0:512],
                            lhsT=yt_all[:, ch, tsl],
                            rhs=wp_all[:, ch, 0:512],
                            start=(ch == 0),
                            stop=(ch == 1),
                        )
                    nc.vector.tensor_copy(out=ot[:, 0:512], in_=pp[:, 0:512])
                    pp2 = ps_big.tile([128, 512], f32, tag="ps")
                    for ch in range(2):
                        nc.tensor.matmul(
                            pp2[:, 0:256],
                            lhsT=yt_all[:, ch, tsl],
                            rhs=wp_all[:, ch, 512:768],
                            start=(ch == 0),
                            stop=(ch == 1),
                        )
                    nc.vector.tensor_copy(out=ot[:, 512:768], in_=pp2[:, 0:256])
                    # alternate out-DMA issuers so the drain isn't
                    # serialized on one HWDGE queue (gpsimd, not scalar:
                    # the Scalar queue paces the exps during attention)
                    out_issuers[r % 2].dma_start(out=out_d[tsl, :], in_=ot[:, :])

            for ci in range(4):
                i_base = 512 * ci
                # v transpose into [token, head-dim] layout + ones column.
                # Emitted here (not in the qkv loop) so the PE's in-order
                # stream doesn't stall on the v1 shift-DMA round trip: by
                # the time group ci's block runs, range ci's shift is long
                # done. Allocated from the qkv-phase psum ring, NOT ps_st:
                # sharing the score-pair ring would serialize attention
                # behind v transposes.
                for tcn in range(4 * ci, 4 * ci + 4):
                    tsl = slice(128 * tcn, 128 * tcn + 128)
                    pv = ps_big.tile([128, 192], bf16, tag="ps")
                    nc.tensor.transpose(
                        pv[:, 0:64], vst[0:64, tsl], id_bf[0:64, 0:64]
                    )
                    nc.tensor.transpose(
                        pv[:, 64:128], v1t[0:64, tsl], id_bf[0:64, 0:64]
                    )
                    nc.tensor.transpose(
                        pv[:, 128:192], v2st[0:64, tsl], id_bf[0:64, 0:64]
                    )
                    vdst = v_all[:, tcn, :].rearrange("p (h e) -> p h e", e=65)
                    nc.vector.tensor_copy(
                        out=vdst[:, :, 0:64],
                        in_=pv[:, 0:192].rearrange("p (h e) -> p h e", e=64),
                    )
                    nc.vector.memset(vdst[:, :, 64:65], 1.0)
                for h in range(HEADS_PER_CORE):
                    k_sl, q_sl = head_ops[h]
                    # score tiles in pairs of j-chunks: two matmuls into one
                    # 2-bank psum, one wide exp (halves ACT instruction count)
                    ets = {}
                    npair = 2 * ci + 2
                    for p in range(npair):
                        ps = ps_st.tile([128, 1024], f32, tag="st")
                        et = exp_pool.tile([128, 1024], bf16, tag=f"p{p}")
                        exp_from = None  # start col of a pending fused exp
                        for half in range(2):
                            jc = 2 * p + half
                            m = jc - 4 * ci
                            i0 = 128 * m if m >= 0 else 0
                            lo = 512 * half
                            nc.tensor.matmul(
                                ps[:, lo + i0 : lo + 512],
                                lhsT=k_sl[:, 128 * jc : 128 * jc + 128],
                                rhs=q_sl[:, i_base + i0 : i_base + 512],
                                start=True,
                                stop=True,
                            )  # K=128 with zero-padded rows 64:128
                            if i0 == 0 and half == 0:
                                exp_from = 0  # may fuse with second half
                            elif i0 == 0 and exp_from == 0:
                                pass  # second half contiguous with first
                            else:
                                if exp_from is not None:
                                    nc.scalar.activation(
                                        et[:, exp_from:lo], ps[:, exp_from:lo], EXP
                                    )
                                exp_from = lo + i0
                            ets[jc] = et
                        nc.scalar.activation(
                            et[:, exp_from:1024], ps[:, exp_from:1024], EXP
                        )
                        for half in range(2):
                            jc = 2 * p + half
                            m = jc - 4 * ci
                            if m >= 0:
                                i0 = 512 * half + 128 * m
                                # block-causal: upper half-block keys masked
                                # for lower half-block queries (DVE: putting
                                # these on Pool makes its in-order queue sit
                                # in waits that delay later Pool work)
                                nc.vector.memset(et[64:128, i0 : i0 + 64], 0.0)

                    if h == 0 and ci > 0:
                        emit_normalize(ci - 1)
                    if h == 1 and ci > 0:
                        emit_proj(ci - 1)

                    # ---- P@V with V stationary: yT[e, i] accumulated over
                    # j-chunks, one 512-col stream per (jc, head). Feature
                    # row 64 is the softmax denominator (ones column) ----
                    py = ps_y.tile([128, 512], f32, tag="py")
                    last = 4 * ci + 3
                    for jc in range(last + 1):
                        m = jc - 4 * ci
                        i0 = 128 * m if m >= 0 else 0
                        lo = 512 * (jc & 1)
                        nc.tensor.matmul(
                            py[0:65, i0:512],
                            lhsT=v_all[:, jc, 65 * h : 65 * h + 65],
                            rhs=ets[jc][:, lo + i0 : lo + 512],
                            start=(jc == 0),
                            stop=(jc == last),
                        )

                    # stage this head's unnormalized yT + denominator row to
                    # SBUF immediately so the psum tile can recycle (the
                    # normalize is deferred to the next group's stream). The
                    # denominator row partition-shifts to a [1, 512] tile
                    # via SBUF->SBUF DMA (DVE copies cannot cross lanes).
                    ys = small.tile([65, 512], f32, tag="ysb", bufs=4)
                    nc.vector.tensor_copy(out=ys, in_=py[0:65, 0:512])
                    ysb_all[(ci, h)] = ys
                    rd = small.tile([1, 512], f32, tag=f"rd{h}", bufs=2)
                    nc.sync.dma_start(out=rd[0:1, :], in_=ys[64:65, :])
                    rd_all[(ci, h)] = rd

            emit_normalize(3)
            emit_proj(3)

    nc.compile()
    return nc


def _get_nc():
    if "nc" not in _CACHE:
        _CACHE["nc"] = _build_bass()
    return _CACHE["nc"]


def make_in_maps(x, c_attn_w, c_proj_w, s):
    import ml_dtypes

    bf16 = ml_dtypes.bfloat16
    x = np.asarray(x, dtype=np.float32)
    c_attn_w = np.asarray(c_attn_w, dtype=np.float32)
    c_proj_w = np.asarray(c_proj_w, dtype=np.float32)
    s = np.asarray(s, dtype=np.float32)

    scale = np.float32(s[0] * np.log(T).astype(np.float32))
    f = np.float32(scale * np.float32(1.0 / np.sqrt(HD)))

    in_maps = []
    for b in range(2):
        xt = np.ascontiguousarray(x[b].T).astype(bf16)  # [768, 2048]
        for g in range(4):
            h0, h1, h2 = 3 * g, 3 * g + 1, 3 * g + 2
            qrow = lambda h: c_attn_w[64 * h : 64 * h + 64] * f  # scaled q
            krow = lambda h: c_attn_w[C + 64 * h : C + 64 * h + 64]
            vrow = lambda h: c_attn_w[2 * C + 64 * h : 2 * C + 64 * h + 64]
            # column order [q0,k0 | q1,k1 | q2,k2 | v0,v1 | v2] (see device side)
            wsel = np.concatenate(
                [
                    qrow(h0), krow(h0),
                    qrow(h1), krow(h1),
                    qrow(h2), krow(h2),
                    vrow(h0), vrow(h1),
                    vrow(h2),
                ],
                axis=0,
            )  # [576, 768]
            wqkv = np.ascontiguousarray(wsel.T).astype(bf16)  # [768, 576]
            wproj = np.zeros((256, C), np.float32)  # rows 192:256 stay zero
            wproj[0:192] = c_proj_w[:, 192 * g : 192 * g + 192].T
            in_maps.append(
                {"xt": xt, "wqkv": wqkv, "wproj": wproj.astype(bf16)}
            )
    return in_maps


def gather(results):
    out = np.empty((2, T, C), dtype=np.float32)
    for b in range(2):
        acc = results[4 * b]["out"].astype(np.float32)
        for g in range(1, 4):
            acc = acc + results[4 * b + g]["out"]
        out[b] = acc
    return out


def kernel(x, c_attn_w, c_proj_w, s):
    from concourse.bass_utils import run_bass_kernel_spmd

    nc = _get_nc()
    in_maps = make_in_maps(x, c_attn_w, c_proj_w, s)
    res = run_bass_kernel_spmd(nc, in_maps, list(range(N_CORES)))
    return gather(res.results)
